# revision 19
# baseline (speedup 1.0000x reference)
# Causal self-attention (B=2, T=2048, D=1024, H=16, HD=64) with RoPE on 8 TRN2 cores.
#
# Sharding: data-parallel over batch (2 groups of 4 cores), tensor-parallel over
# heads within each group (4 heads per core). Each core computes, for its batch b
# and its 4 heads:
#   qkv^T projection (fp32r matmuls), RoPE on q/k, causal attention in a
#   transposed (S^T) layout with exp on the Scalar engine, AV with an augmented
#   ones-column producing the softmax denominator for free, and a row-sharded
#   out-projection producing a partial [D, T] output. The host sums the 4
#   partials per batch and transposes back.
#
# Everything on the PE runs in float32r (~13-bit mantissa, full speed at
# moving-dim >= 256). No max-subtraction in softmax: logits are ~N(0,1) here,
# exp never overflows.
import sys
import os

sys.path.insert(0, "/opt/trn_rl_repo")

import numpy as np

import concourse.bass as bass  # noqa: F401  (bass types used via bacc)
import concourse.mybir as mybir
from concourse import bacc
from concourse.tile import TileContext
from concourse.bass_utils import run_bass_kernel_spmd
from contextlib import ExitStack

F32 = mybir.dt.float32
F32R = mybir.dt.float32r
BF16 = mybir.dt.bfloat16
AF = mybir.ActivationFunctionType
ALU = mybir.AluOpType

B, T, D = 2, 2048, 1024
H, HD = 16, 64
NCORES = 8
GROUPS = NCORES // B          # cores per batch = 4
HPC = H // GROUPS             # heads per core = 4
NK = D // 128                 # contraction tiles for D
SCALE = HD ** -0.5

# hd interleave: new row 2j <- orig j, new row 2j+1 <- orig j+32 so the
# rotate-half partner of every row is its neighbour (swappable by a 32-lane
# stream shuffle).
PI = np.empty(HD, dtype=np.int64)
PI[0::2] = np.arange(32)
PI[1::2] = np.arange(32, 64)

SWAP_MASK = []
for _i in range(16):
    SWAP_MASK += [2 * _i + 1, 2 * _i]


def _sq_chunks(o, end=1024):
    """Chunks [pos, pos+cl) from o to end that never cross a 512-aligned PSUM
    bank boundary (a single matmul output must stay inside one bank)."""
    out = []
    pos = o
    while pos < end:
        nxt = min(end, (pos // 512 + 1) * 512)
        out.append((pos, nxt - pos))
        pos = nxt
    return out


def _build_program():
    nc = bacc.Bacc("TRN2", target_bir_lowering=False, debug=False,
                   num_devices=NCORES)
    d_xT = nc.dram_tensor("xT", [D, T], F32, kind="ExternalInput").ap()
    d_w = nc.dram_tensor("w_cat", [D, 6 * 128], F32, kind="ExternalInput").ap()
    d_wo = nc.dram_tensor("w_o", [2 * 128, D], F32, kind="ExternalInput").ap()
    d_cos = nc.dram_tensor("cos2", [128, T], F32, kind="ExternalInput").ap()
    d_sin = nc.dram_tensor("sin2", [128, T], F32, kind="ExternalInput").ap()
    d_id = nc.dram_tensor("ident", [128, 128], F32, kind="ExternalInput").ap()
    d_ones = nc.dram_tensor("ones16", [128, 16], F32, kind="ExternalInput").ap()
    d_out = nc.dram_tensor("outp", [D, T], BF16, kind="ExternalOutput").ap()
    dbg = bool(int(os.environ.get("KDEBUG", "0")))
    if dbg:
        d_dbg_q0 = nc.dram_tensor("dbg_q0", [128, T], F32, kind="ExternalOutput").ap()
        d_dbg_k0 = nc.dram_tensor("dbg_k0", [128, T], F32, kind="ExternalOutput").ap()
        d_dbg_va0 = nc.dram_tensor("dbg_va0", [128, 16 * 65], F32, kind="ExternalOutput").ap()
        d_dbg_o0 = nc.dram_tensor("dbg_o0", [128, T], F32, kind="ExternalOutput").ap()

    with TileContext(nc) as tc, nc.allow_low_precision(reason="f32r attention"):
        with ExitStack() as root:
            qkv_pool = root.enter_context(tc.tile_pool(name="qkv", bufs=1))
            va_pool = root.enter_context(tc.tile_pool(name="va", bufs=1))
            out_pool = root.enter_context(tc.tile_pool(name="outT", bufs=1))
            wop = root.enter_context(tc.tile_pool(name="wop", bufs=1))

            qT = [qkv_pool.tile([128, T], F32R, tag=f"q{p}", name=f"qT{p}")
                  for p in range(2)]
            kT = [qkv_pool.tile([128, T], F32R, tag=f"k{p}", name=f"kTt{p}")
                  for p in range(2)]
            va = [va_pool.tile([128, 16 * 65], F32R, tag=f"va{h}",
                               name=f"va{h}") for h in range(HPC)]
            oT = [out_pool.tile([128, T], F32R, tag=f"o{p}", name=f"oT{p}")
                  for p in range(2)]
            wo_sb = [wop.tile([128, D], F32R, tag=f"wo{p}", name=f"wo{p}")
                     for p in range(2)]
            for p in range(2):
                nc.scalar.dma_start(
                    out=wo_sb[p][:],
                    in_=d_wo[p * 128:(p + 1) * 128, :].bitcast(F32R))

            # ---------------- Phase A: qkv^T projection + RoPE + v transpose
            with nc.named_scope("qkv"):
                with ExitStack() as sA:
                    tab = sA.enter_context(tc.tile_pool(name="tab", bufs=1))
                    xp = sA.enter_context(tc.tile_pool(name="xp", bufs=1))
                    wp = sA.enter_context(tc.tile_pool(name="wp", bufs=4))
                    tp = sA.enter_context(tc.tile_pool(name="ropetmp", bufs=1))
                    vtp = sA.enter_context(tc.tile_pool(name="vT", bufs=1))

                    cos2 = tab.tile([128, T], F32, tag="cos")
                    sin2 = tab.tile([128, T], F32, tag="sin")
                    ident = tab.tile([128, 128], F32R, tag="id")
                    nc.scalar.dma_start(out=cos2[:], in_=d_cos[:])
                    nc.scalar.dma_start(out=sin2[:], in_=d_sin[:])
                    nc.scalar.dma_start(out=ident[:], in_=d_id[:].bitcast(F32R))

                    x_sb = []
                    for kt in range(NK):
                        t_ = xp.tile([128, T], F32R, tag=f"x{kt}",
                                     name=f"xsb{kt}")
                        nc.sync.dma_start(
                            out=t_[:],
                            in_=d_xT[kt * 128:(kt + 1) * 128, :].bitcast(F32R))
                        x_sb.append(t_)

                    vT = [vtp.tile([128, T], F32R, tag=f"v{p}", name=f"vT{p}")
                          for p in range(2)]
                    qsh = tp.tile([128, T], F32, tag="qsh")
                    tcos = tp.tile([128, T], F32, tag="tcos")

                    def emit_proj(c, psum_pool, tag_prefix):
                        pc = []
                        for t in range(4):
                            pc.append(psum_pool.tile(
                                [128, 512], F32, tag=f"{tag_prefix}{t}",
                                name=f"pc{c}_{t}"))
                        for kt in range(NK):
                            w_t = wp.tile([128, 128], F32R, tag="w")
                            nc.scalar.dma_start(
                                out=w_t[:],
                                in_=d_w[kt * 128:(kt + 1) * 128,
                                        c * 128:(c + 1) * 128].bitcast(F32R))
                            for t in range(4):
                                nc.tensor.matmul(
                                    pc[t][:], w_t[:],
                                    x_sb[kt][:, t * 512:(t + 1) * 512],
                                    start=(kt == 0), stop=(kt == NK - 1))
                        return pc

                    def emit_rope(c, pc):
                        dst = qT[c - 2] if c < 4 else kT[c - 4]
                        for t in range(4):
                            sl = slice(t * 512, (t + 1) * 512)
                            nc.vector.stream_shuffle(qsh[:, sl], pc[t][:],
                                                     SWAP_MASK)
                            nc.vector.tensor_tensor(
                                out=tcos[:, sl], in0=pc[t][:],
                                in1=cos2[:, sl], op=ALU.mult)
                        nc.vector.tensor_tensor(out=qsh[:], in0=qsh[:],
                                                in1=sin2[:], op=ALU.mult)
                        nc.vector.tensor_tensor(out=dst[:], in0=qsh[:],
                                                in1=tcos[:], op=ALU.add)

                    # q0/k0 first (left PSUM stack) so pair-0 attention can
                    # begin while pair 1 is still projecting; v + transposes on
                    # the right stack.
                    psQK = tc.alloc_tile_pool(name="psQK", bufs=1,
                                              space="PSUM")
                    pc = emit_proj(2, psQK, "paq")
                    emit_rope(2, pc)
                    pc = emit_proj(4, psQK, "paq")
                    emit_rope(4, pc)

                    psAv = tc.alloc_tile_pool(name="psAv", bufs=1,
                                              space="PSUM", side="right")
                    for c in range(2):
                        pc = emit_proj(c, psAv, "pav")
                        for t in range(4):
                            nc.any.tensor_copy(
                                vT[c][:, t * 512:(t + 1) * 512], pc[t][:])
                    psAv.release()
                    for h in range(HPC):
                        nc.sync.dma_start(out=va[h][:, 64:16 * 65:65],
                                          in_=d_ones[:].bitcast(F32R))
                    psT = tc.alloc_tile_pool(name="psT", bufs=4, space="PSUM",
                                             side="right")
                    for p in range(2):
                        for tt in range(16):
                            pt_ = psT.tile([128, 128], F32R, tag="pt",
                                           name=f"ptr{p}_{tt}")
                            nc.tensor.transpose(
                                pt_[:], vT[p][:, tt * 128:(tt + 1) * 128],
                                ident[:])
                            nc.any.tensor_copy(
                                va[2 * p][:, tt * 65:tt * 65 + 64],
                                pt_[:, 0:64])
                            nc.any.tensor_copy(
                                va[2 * p + 1][:, tt * 65:tt * 65 + 64],
                                pt_[:, 64:128])
                    psT.release()
                    # psS takes the right-side banks; pair-0 S/exp overlaps the
                    # pair-1 projection below.
                    psS = tc.alloc_tile_pool(name="psS", bufs=2, space="PSUM",
                                             side="right")
                    pc = emit_proj(3, psQK, "paq")
                    emit_rope(3, pc)
                    pc = emit_proj(5, psQK, "paq")
                    emit_rope(5, pc)
                    psQK.release()

            psV = tc.alloc_tile_pool(name="psV", bufs=2, space="PSUM")

            # ---------------- Phase B/C: causal attention, q-strips of 512
            with nc.named_scope("attn"):
                with ExitStack() as sB:
                    ptp = sB.enter_context(tc.tile_pool(name="ptp", bufs=5))
                    rp = sB.enter_context(tc.tile_pool(name="rp", bufs=2))

                    for si in range(4):
                        q0 = 512 * si
                        kb_max = 4 * (si + 1)
                        for p in range(2):
                            av = [psV.tile([65, 512], F32, tag=f"av{hl}",
                                           name=f"avps{si}_{p}_{hl}")
                                  for hl in range(2)]
                            for kb in range(kb_max):
                                o = max(0, 128 * kb - q0)
                                # S^T for both heads into one [128, 1024]
                                # psum tile (head hl at cols hl*512+...)
                                sps = psS.tile([128, 1024], F32, tag="sps",
                                               name=f"sps{si}_{p}_{kb}")
                                for hl in range(2):
                                    hb = 64 * hl
                                    for pos, cl in _sq_chunks(o, 512):
                                        nc.tensor.matmul(
                                            sps[:, 512 * hl + pos:
                                                512 * hl + pos + cl],
                                            kT[p][hb:hb + 64,
                                                  kb * 128:(kb + 1) * 128],
                                            qT[p][hb:hb + 64,
                                                  q0 + pos:q0 + pos + cl],
                                            start=True, stop=True)
                                ptb = ptp.tile([128, 1024], F32R, tag="ptb",
                                               name=f"ptb{si}_{p}_{kb}")
                                L = 512 - o
                                sps3 = sps[:].rearrange(
                                    "a (h q) -> a h q", h=2)
                                ptb3 = ptb[:].rearrange(
                                    "a (h q) -> a h q", h=2)
                                nc.scalar.activation(
                                    ptb3[:, :, 0:L], sps3[:, :, o:512],
                                    AF.Exp, scale=SCALE)
                                if 128 * kb >= q0:
                                    for hl in range(2):
                                        nc.gpsimd.affine_select(
                                            ptb[:, 512 * hl:512 * hl + 128],
                                            ptb[:, 512 * hl:512 * hl + 128],
                                            pattern=[[1, 128]],
                                            compare_op=ALU.is_ge, fill=0.0,
                                            base=0, channel_multiplier=-1)
                                for hl in range(2):
                                    h = 2 * p + hl
                                    for pos, cl in _sq_chunks(o, 512):
                                        nc.tensor.matmul(
                                            av[hl][:, pos:pos + cl],
                                            va[h][:, kb * 65:kb * 65 + 65],
                                            ptb[:, 512 * hl + pos - o:
                                                512 * hl + pos - o + cl],
                                            start=(kb == 0),
                                            stop=(kb == kb_max - 1),
                                            skip_group_check=True)
                            for hl in range(2):
                                r_sb = rp.tile([1, 512], F32, tag=f"r{hl}",
                                               name=f"rsb{si}_{p}_{hl}")
                                nc.vector.reciprocal(r_sb[:],
                                                     av[hl][64:65, :])
                                rb = rp.tile([64, 512], F32, tag=f"rb{hl}",
                                             name=f"rbb{si}_{p}_{hl}")
                                nc.gpsimd.partition_broadcast(rb[:], r_sb[:])
                                nc.vector.tensor_tensor(
                                    out=oT[p][64 * hl:64 * hl + 64,
                                              q0:q0 + 512],
                                    in0=av[hl][0:64, :], in1=rb[:],
                                    op=ALU.mult)

            psV.release()
            psS.release()

            if dbg:
                nc.sync.dma_start(out=d_dbg_q0[:], in_=qT[0][:].bitcast(F32))
                nc.sync.dma_start(out=d_dbg_k0[:], in_=kT[0][:].bitcast(F32))
                nc.sync.dma_start(out=d_dbg_va0[:], in_=va[0][:].bitcast(F32))
                nc.sync.dma_start(out=d_dbg_o0[:], in_=oT[0][:].bitcast(F32))

            # ---------------- Phase D: out-projection (row-sharded, partial)
            with nc.named_scope("oproj"):
                with ExitStack() as sD:
                    fop = sD.enter_context(tc.tile_pool(name="fop", bufs=4))
                    psD = sD.enter_context(
                        tc.tile_pool(name="psD", bufs=1, space="PSUM"))
                    for t in range(4):
                        pD = [psD.tile([128, 512], F32, tag=f"pd{n}",
                                       name=f"pD{t}_{n}") for n in range(8)]
                        for p in range(2):
                            for n in range(8):
                                nc.tensor.matmul(
                                    pD[n][:],
                                    wo_sb[p][:, n * 128:(n + 1) * 128],
                                    oT[p][:, t * 512:(t + 1) * 512],
                                    start=(p == 0), stop=(p == 1))
                        for n in range(8):
                            fo = fop.tile([128, 512], BF16, tag="fo")
                            nc.any.tensor_copy(fo[:], pD[n][:])
                            nc.sync.dma_start(
                                out=d_out[n * 128:(n + 1) * 128,
                                          t * 512:(t + 1) * 512],
                                in_=fo[:])

    nc.compile()
    return nc


_NC_CACHE = None


def _get_program():
    global _NC_CACHE
    if _NC_CACHE is None:
        _NC_CACHE = _build_program()
    return _NC_CACHE


def _rope_tables():
    inv_freq = 1.0 / (10000.0 ** (np.arange(0, HD, 2, dtype=np.float32) / HD))
    freqs = np.outer(np.arange(T, dtype=np.float32), inv_freq)  # [T, 32]
    emb = np.concatenate([freqs, freqs], axis=-1)               # [T, 64]
    return np.cos(emb), np.sin(emb)


def _host_prep(x, w_qkv, w_out):
    cos, sin = _rope_tables()          # [T, 64] each, original hd order
    # permuted + transposed tables [64, T], duplicated for a 2-head pair tile
    cosP = np.ascontiguousarray(cos.T[PI, :])                   # [64, T]
    sinP = sin.T[PI, :].copy()                                  # [64, T]
    sinP[0::2, :] *= -1.0                                       # sign baked in
    cos2 = np.ascontiguousarray(np.vstack([cosP, cosP]), dtype=np.float32)
    sin2 = np.ascontiguousarray(np.vstack([sinP, sinP]), dtype=np.float32)
    ident = np.eye(128, dtype=np.float32)

    in_maps = []
    for core in range(NCORES):
        b = core // GROUPS
        h0 = (core % GROUPS) * HPC
        xT = np.ascontiguousarray(x[b].T)                       # [D, T]
        cols = []
        for p in range(2):                                      # v (no perm)
            for hh in range(2):
                h = h0 + 2 * p + hh
                cols.append(w_qkv[:, 2 * D + h * HD:2 * D + (h + 1) * HD])
        for kind in range(2):                                   # q, k
            for p in range(2):                                  # head pairs
                for hh in range(2):
                    h = h0 + 2 * p + hh
                    wcol = w_qkv[:, kind * D + h * HD:kind * D + (h + 1) * HD]
                    cols.append(wcol[:, PI])
        w_cat = np.ascontiguousarray(np.concatenate(cols, axis=1),
                                     dtype=np.float32)          # [D, 768]
        w_o = np.ascontiguousarray(
            w_out[h0 * HD:(h0 + HPC) * HD, :], dtype=np.float32)  # [256, D]
        in_maps.append({
            "xT": xT.astype(np.float32, copy=False),
            "w_cat": w_cat,
            "w_o": w_o,
            "cos2": cos2,
            "sin2": sin2,
            "ident": ident,
            "ones16": np.ones((128, 16), dtype=np.float32),
        })
    return in_maps


def kernel(x, w_qkv, w_out):
    x = np.asarray(x, dtype=np.float32)
    w_qkv = np.asarray(w_qkv, dtype=np.float32)
    w_out = np.asarray(w_out, dtype=np.float32)
    nc = _get_program()
    in_maps = _host_prep(x, w_qkv, w_out)
    trace = bool(int(os.environ.get("KBENCH_TRACE", "0")))
    res = run_bass_kernel_spmd(nc, in_maps, list(range(NCORES)), trace=trace)
    if trace and res.exec_time_ns is not None:
        print(f"HW exec time: {res.exec_time_ns} ns")
        if res.per_core_scope_times:
            for scope, cores in sorted(res.per_core_scope_times.items()):
                print(f"  scope {scope}: {cores}")
    out = np.zeros((B, T, D), dtype=np.float32)
    for core in range(NCORES):
        b = core // GROUPS
        out[b] += res.results[core]["outp"].T.astype(np.float32)
    return out


# revision 25
# speedup vs baseline: 69.5476x; 69.5476x over previous
# Causal self-attention (B=2, T=2048, D=1024, H=16, HD=64) with RoPE on 8 TRN2 cores.
#
# Sharding: data-parallel over batch (2 groups of 4 cores), tensor-parallel over
# heads within each group (4 heads per core). Each core computes, for its batch b
# and its 4 heads:
#   qkv^T projection (fp32r matmuls), RoPE on q/k, causal attention in a
#   transposed (S^T) layout with exp on the Scalar engine, AV with an augmented
#   ones-column producing the softmax denominator for free, and a row-sharded
#   out-projection producing a partial [D, T] output. The host sums the 4
#   partials per batch and transposes back.
#
# Everything on the PE runs in float32r (~13-bit mantissa, full speed at
# moving-dim >= 256). No max-subtraction in softmax: logits are ~N(0,1) here,
# exp never overflows.
import sys
import os

sys.path.insert(0, "/opt/trn_rl_repo")

import numpy as np

import concourse.bass as bass  # noqa: F401  (bass types used via bacc)
import concourse.mybir as mybir
from concourse import bacc
from concourse.tile import TileContext
from concourse.bass_utils import run_bass_kernel_spmd
from contextlib import ExitStack

F32 = mybir.dt.float32
F32R = mybir.dt.float32r
BF16 = mybir.dt.bfloat16
AF = mybir.ActivationFunctionType
ALU = mybir.AluOpType

B, T, D = 2, 2048, 1024
H, HD = 16, 64
NCORES = 8
GROUPS = NCORES // B          # cores per batch = 4
HPC = H // GROUPS             # heads per core = 4
NK = D // 128                 # contraction tiles for D
SCALE = HD ** -0.5

# hd interleave: new row 2j <- orig j, new row 2j+1 <- orig j+32 so the
# rotate-half partner of every row is its neighbour (swappable by a 32-lane
# stream shuffle).
PI = np.empty(HD, dtype=np.int64)
PI[0::2] = np.arange(32)
PI[1::2] = np.arange(32, 64)

SWAP_MASK = []
for _i in range(16):
    SWAP_MASK += [2 * _i + 1, 2 * _i]


def _sq_chunks(o, end=1024):
    """Chunks [pos, pos+cl) from o to end that never cross a 512-aligned PSUM
    bank boundary (a single matmul output must stay inside one bank)."""
    out = []
    pos = o
    while pos < end:
        nxt = min(end, (pos // 512 + 1) * 512)
        out.append((pos, nxt - pos))
        pos = nxt
    return out


def _build_program():
    nc = bacc.Bacc("TRN2", target_bir_lowering=False, debug=False,
                   num_devices=NCORES)
    d_xT = nc.dram_tensor("xT", [D, T], F32, kind="ExternalInput").ap()
    d_w = nc.dram_tensor("w_cat", [D, 6 * 128], F32, kind="ExternalInput").ap()
    d_wo = nc.dram_tensor("w_o", [2 * 128, D], F32, kind="ExternalInput").ap()
    d_cos = nc.dram_tensor("cos2", [128, T], F32, kind="ExternalInput").ap()
    d_sin = nc.dram_tensor("sin2", [128, T], F32, kind="ExternalInput").ap()
    d_id = nc.dram_tensor("ident", [128, 128], F32, kind="ExternalInput").ap()
    d_ones = nc.dram_tensor("ones16", [128, 16], F32, kind="ExternalInput").ap()
    d_out = nc.dram_tensor("outp", [D, T], BF16, kind="ExternalOutput").ap()
    dbg = bool(int(os.environ.get("KDEBUG", "0")))
    if dbg:
        d_dbg_q0 = nc.dram_tensor("dbg_q0", [128, T], F32, kind="ExternalOutput").ap()
        d_dbg_k0 = nc.dram_tensor("dbg_k0", [128, T], F32, kind="ExternalOutput").ap()
        d_dbg_va0 = nc.dram_tensor("dbg_va0", [128, 16 * 65], F32, kind="ExternalOutput").ap()
        d_dbg_o0 = nc.dram_tensor("dbg_o0", [128, T], F32, kind="ExternalOutput").ap()

    with TileContext(nc) as tc, nc.allow_low_precision(reason="f32r attention"):
        with ExitStack() as root:
            qkv_pool = root.enter_context(tc.tile_pool(name="qkv", bufs=1))
            va_pool = root.enter_context(tc.tile_pool(name="va", bufs=1))
            out_pool = root.enter_context(tc.tile_pool(name="outT", bufs=1))
            wop = root.enter_context(tc.tile_pool(name="wop", bufs=1))

            qT = [qkv_pool.tile([128, T], F32R, tag=f"q{p}", name=f"qT{p}")
                  for p in range(2)]
            kT = [qkv_pool.tile([128, T], F32R, tag=f"k{p}", name=f"kTt{p}")
                  for p in range(2)]
            va = [va_pool.tile([128, 16 * 65], F32R, tag=f"va{h}",
                               name=f"va{h}") for h in range(HPC)]
            oT = [out_pool.tile([128, T], F32R, tag=f"o{p}", name=f"oT{p}")
                  for p in range(2)]
            wo_sb = [wop.tile([128, D], F32R, tag=f"wo{p}", name=f"wo{p}")
                     for p in range(2)]
            for p in range(2):
                nc.scalar.dma_start(
                    out=wo_sb[p][:],
                    in_=d_wo[p * 128:(p + 1) * 128, :].bitcast(F32R))

            # ---------------- Phase A: qkv^T projection + RoPE + v transpose
            with nc.named_scope("qkv"):
                with ExitStack() as sA:
                    tab = sA.enter_context(tc.tile_pool(name="tab", bufs=1))
                    xp = sA.enter_context(tc.tile_pool(name="xp", bufs=1))
                    wp = sA.enter_context(tc.tile_pool(name="wp", bufs=4))
                    tp = sA.enter_context(tc.tile_pool(name="ropetmp", bufs=1))
                    vtp = sA.enter_context(tc.tile_pool(name="vT", bufs=1))

                    cos2 = tab.tile([128, T], F32, tag="cos")
                    sin2 = tab.tile([128, T], F32, tag="sin")
                    ident = tab.tile([128, 128], F32R, tag="id")
                    nc.scalar.dma_start(out=cos2[:], in_=d_cos[:])
                    nc.scalar.dma_start(out=sin2[:], in_=d_sin[:])
                    nc.scalar.dma_start(out=ident[:], in_=d_id[:].bitcast(F32R))

                    x_sb = []
                    for kt in range(NK):
                        t_ = xp.tile([128, T], F32R, tag=f"x{kt}",
                                     name=f"xsb{kt}")
                        nc.sync.dma_start(
                            out=t_[:],
                            in_=d_xT[kt * 128:(kt + 1) * 128, :].bitcast(F32R))
                        x_sb.append(t_)

                    vT = [vtp.tile([128, T], F32R, tag=f"v{p}", name=f"vT{p}")
                          for p in range(2)]
                    qsh = tp.tile([128, T], F32, tag="qsh")
                    tcos = tp.tile([128, T], F32, tag="tcos")

                    def emit_proj(c, psum_pool, tag_prefix):
                        pc = []
                        for t in range(4):
                            pc.append(psum_pool.tile(
                                [128, 512], F32, tag=f"{tag_prefix}{t}",
                                name=f"pc{c}_{t}"))
                        for kt in range(NK):
                            w_t = wp.tile([128, 128], F32R, tag="w")
                            nc.scalar.dma_start(
                                out=w_t[:],
                                in_=d_w[kt * 128:(kt + 1) * 128,
                                        c * 128:(c + 1) * 128].bitcast(F32R))
                            for t in range(4):
                                nc.tensor.matmul(
                                    pc[t][:], w_t[:],
                                    x_sb[kt][:, t * 512:(t + 1) * 512],
                                    start=(kt == 0), stop=(kt == NK - 1))
                        return pc

                    def emit_rope(c, pc):
                        dst = qT[c - 2] if c < 4 else kT[c - 4]
                        for t in range(4):
                            sl = slice(t * 512, (t + 1) * 512)
                            nc.vector.stream_shuffle(qsh[:, sl], pc[t][:],
                                                     SWAP_MASK)
                            nc.vector.tensor_tensor(
                                out=tcos[:, sl], in0=pc[t][:],
                                in1=cos2[:, sl], op=ALU.mult)
                        nc.vector.tensor_tensor(out=qsh[:], in0=qsh[:],
                                                in1=sin2[:], op=ALU.mult)
                        nc.vector.tensor_tensor(out=dst[:], in0=qsh[:],
                                                in1=tcos[:], op=ALU.add)

                    # q0/k0 first (left PSUM stack) so pair-0 attention can
                    # begin while pair 1 is still projecting; v + transposes on
                    # the right stack.
                    psQK = tc.alloc_tile_pool(name="psQK", bufs=1,
                                              space="PSUM")
                    pc = emit_proj(2, psQK, "paq")
                    emit_rope(2, pc)
                    pc = emit_proj(4, psQK, "paq")
                    emit_rope(4, pc)

                    psAv = tc.alloc_tile_pool(name="psAv", bufs=1,
                                              space="PSUM", side="right")
                    for c in range(2):
                        pc = emit_proj(c, psAv, "pav")
                        for t in range(4):
                            nc.any.tensor_copy(
                                vT[c][:, t * 512:(t + 1) * 512], pc[t][:])
                    psAv.release()
                    for h in range(HPC):
                        nc.sync.dma_start(out=va[h][:, 64:16 * 65:65],
                                          in_=d_ones[:].bitcast(F32R))
                    psT = tc.alloc_tile_pool(name="psT", bufs=4, space="PSUM",
                                             side="right")
                    for p in range(2):
                        for tt in range(16):
                            pt_ = psT.tile([128, 128], F32R, tag="pt",
                                           name=f"ptr{p}_{tt}")
                            nc.tensor.transpose(
                                pt_[:], vT[p][:, tt * 128:(tt + 1) * 128],
                                ident[:])
                            nc.any.tensor_copy(
                                va[2 * p][:, tt * 65:tt * 65 + 64],
                                pt_[:, 0:64])
                            nc.any.tensor_copy(
                                va[2 * p + 1][:, tt * 65:tt * 65 + 64],
                                pt_[:, 64:128])
                    psT.release()
                    # psS takes the right-side banks; pair-0 S/exp overlaps the
                    # pair-1 projection below.
                    psS = tc.alloc_tile_pool(name="psS", bufs=2, space="PSUM",
                                             side="right")
                    pc = emit_proj(3, psQK, "paq")
                    emit_rope(3, pc)
                    pc = emit_proj(5, psQK, "paq")
                    emit_rope(5, pc)
                    psQK.release()

            psV = tc.alloc_tile_pool(name="psV", bufs=2, space="PSUM")

            # ---------------- Phase B/C: causal attention, q-strips of 512
            with nc.named_scope("attn"):
                with ExitStack() as sB:
                    ptp = sB.enter_context(tc.tile_pool(name="ptp", bufs=5))
                    rp = sB.enter_context(tc.tile_pool(name="rp", bufs=2))

                    for si in range(4):
                        q0 = 512 * si
                        kb_max = 4 * (si + 1)
                        for p in range(2):
                            av = [psV.tile([65, 512], F32, tag=f"av{hl}",
                                           name=f"avps{si}_{p}_{hl}")
                                  for hl in range(2)]
                            for kb in range(kb_max):
                                o = max(0, 128 * kb - q0)
                                # S^T for both heads into one [128, 1024]
                                # psum tile (head hl at cols hl*512+...)
                                sps = psS.tile([128, 1024], F32, tag="sps",
                                               name=f"sps{si}_{p}_{kb}")
                                for hl in range(2):
                                    hb = 64 * hl
                                    for pos, cl in _sq_chunks(o, 512):
                                        nc.tensor.matmul(
                                            sps[:, 512 * hl + pos:
                                                512 * hl + pos + cl],
                                            kT[p][hb:hb + 64,
                                                  kb * 128:(kb + 1) * 128],
                                            qT[p][hb:hb + 64,
                                                  q0 + pos:q0 + pos + cl],
                                            start=True, stop=True)
                                ptb = ptp.tile([128, 1024], F32R, tag="ptb",
                                               name=f"ptb{si}_{p}_{kb}")
                                L = 512 - o
                                sps3 = sps[:].rearrange(
                                    "a (h q) -> a h q", h=2)
                                ptb3 = ptb[:].rearrange(
                                    "a (h q) -> a h q", h=2)
                                nc.scalar.activation(
                                    ptb3[:, :, 0:L], sps3[:, :, o:512],
                                    AF.Exp, scale=SCALE)
                                if 128 * kb >= q0:
                                    for hl in range(2):
                                        nc.gpsimd.affine_select(
                                            ptb[:, 512 * hl:512 * hl + 128],
                                            ptb[:, 512 * hl:512 * hl + 128],
                                            pattern=[[1, 128]],
                                            compare_op=ALU.is_ge, fill=0.0,
                                            base=0, channel_multiplier=-1)
                                for hl in range(2):
                                    h = 2 * p + hl
                                    for pos, cl in _sq_chunks(o, 512):
                                        nc.tensor.matmul(
                                            av[hl][:, pos:pos + cl],
                                            va[h][:, kb * 65:kb * 65 + 65],
                                            ptb[:, 512 * hl + pos - o:
                                                512 * hl + pos - o + cl],
                                            start=(kb == 0),
                                            stop=(kb == kb_max - 1),
                                            skip_group_check=True)
                            for hl in range(2):
                                r_sb = rp.tile([1, 512], F32, tag=f"r{hl}",
                                               name=f"rsb{si}_{p}_{hl}")
                                nc.vector.reciprocal(r_sb[:],
                                                     av[hl][64:65, :])
                                rb = rp.tile([64, 512], F32, tag=f"rb{hl}",
                                             name=f"rbb{si}_{p}_{hl}")
                                nc.gpsimd.partition_broadcast(rb[:], r_sb[:])
                                nc.vector.tensor_tensor(
                                    out=oT[p][64 * hl:64 * hl + 64,
                                              q0:q0 + 512],
                                    in0=av[hl][0:64, :], in1=rb[:],
                                    op=ALU.mult)

            psV.release()
            psS.release()

            if dbg:
                nc.sync.dma_start(out=d_dbg_q0[:], in_=qT[0][:].bitcast(F32))
                nc.sync.dma_start(out=d_dbg_k0[:], in_=kT[0][:].bitcast(F32))
                nc.sync.dma_start(out=d_dbg_va0[:], in_=va[0][:].bitcast(F32))
                nc.sync.dma_start(out=d_dbg_o0[:], in_=oT[0][:].bitcast(F32))

            # ---------------- Phase D: out-projection (row-sharded, partial)
            with nc.named_scope("oproj"):
                with ExitStack() as sD:
                    fop = sD.enter_context(tc.tile_pool(name="fop", bufs=4))
                    psD = sD.enter_context(
                        tc.tile_pool(name="psD", bufs=1, space="PSUM"))
                    for t in range(4):
                        pD = [psD.tile([128, 512], F32, tag=f"pd{n}",
                                       name=f"pD{t}_{n}") for n in range(8)]
                        for p in range(2):
                            for n in range(8):
                                nc.tensor.matmul(
                                    pD[n][:],
                                    wo_sb[p][:, n * 128:(n + 1) * 128],
                                    oT[p][:, t * 512:(t + 1) * 512],
                                    start=(p == 0), stop=(p == 1))
                        for n in range(8):
                            fo = fop.tile([128, 512], BF16, tag="fo")
                            nc.vector.tensor_copy(fo[:], pD[n][:])
                            nc.sync.dma_start(
                                out=d_out[n * 128:(n + 1) * 128,
                                          t * 512:(t + 1) * 512],
                                in_=fo[:])

    nc.compile()
    return nc


_NC_CACHE = None


def _get_program():
    global _NC_CACHE
    if _NC_CACHE is None:
        _NC_CACHE = _build_program()
    return _NC_CACHE


def _rope_tables():
    inv_freq = 1.0 / (10000.0 ** (np.arange(0, HD, 2, dtype=np.float32) / HD))
    freqs = np.outer(np.arange(T, dtype=np.float32), inv_freq)  # [T, 32]
    emb = np.concatenate([freqs, freqs], axis=-1)               # [T, 64]
    return np.cos(emb), np.sin(emb)


def _host_prep(x, w_qkv, w_out):
    cos, sin = _rope_tables()          # [T, 64] each, original hd order
    # permuted + transposed tables [64, T], duplicated for a 2-head pair tile
    cosP = np.ascontiguousarray(cos.T[PI, :])                   # [64, T]
    sinP = sin.T[PI, :].copy()                                  # [64, T]
    sinP[0::2, :] *= -1.0                                       # sign baked in
    cos2 = np.ascontiguousarray(np.vstack([cosP, cosP]), dtype=np.float32)
    sin2 = np.ascontiguousarray(np.vstack([sinP, sinP]), dtype=np.float32)
    ident = np.eye(128, dtype=np.float32)

    in_maps = []
    for core in range(NCORES):
        b = core // GROUPS
        h0 = (core % GROUPS) * HPC
        xT = np.ascontiguousarray(x[b].T)                       # [D, T]
        cols = []
        for p in range(2):                                      # v (no perm)
            for hh in range(2):
                h = h0 + 2 * p + hh
                cols.append(w_qkv[:, 2 * D + h * HD:2 * D + (h + 1) * HD])
        for kind in range(2):                                   # q, k
            for p in range(2):                                  # head pairs
                for hh in range(2):
                    h = h0 + 2 * p + hh
                    wcol = w_qkv[:, kind * D + h * HD:kind * D + (h + 1) * HD]
                    cols.append(wcol[:, PI])
        w_cat = np.ascontiguousarray(np.concatenate(cols, axis=1),
                                     dtype=np.float32)          # [D, 768]
        w_o = np.ascontiguousarray(
            w_out[h0 * HD:(h0 + HPC) * HD, :], dtype=np.float32)  # [256, D]
        in_maps.append({
            "xT": xT.astype(np.float32, copy=False),
            "w_cat": w_cat,
            "w_o": w_o,
            "cos2": cos2,
            "sin2": sin2,
            "ident": ident,
            "ones16": np.ones((128, 16), dtype=np.float32),
        })
    return in_maps


def kernel(x, w_qkv, w_out):
    x = np.asarray(x, dtype=np.float32)
    w_qkv = np.asarray(w_qkv, dtype=np.float32)
    w_out = np.asarray(w_out, dtype=np.float32)
    nc = _get_program()
    in_maps = _host_prep(x, w_qkv, w_out)
    trace = bool(int(os.environ.get("KBENCH_TRACE", "0")))
    res = run_bass_kernel_spmd(nc, in_maps, list(range(NCORES)), trace=trace)
    if trace and res.exec_time_ns is not None:
        print(f"HW exec time: {res.exec_time_ns} ns")
        if res.per_core_scope_times:
            for scope, cores in sorted(res.per_core_scope_times.items()):
                print(f"  scope {scope}: {cores}")
    out = np.zeros((B, T, D), dtype=np.float32)
    for core in range(NCORES):
        b = core // GROUPS
        out[b] += res.results[core]["outp"].T.astype(np.float32)
    return out
